# revision 73
# baseline (speedup 1.0000x reference)
"""Causal self-attention (B=2, T=2048, C=768, H=12, DH=64) on 8 TRN2 cores.

Sharding: core = (b, g) with b in {0,1} batch, g in {0..3} head-group of 3
heads.  Data parallel on B, tensor parallel on H: Wqkv column-sharded,
Wout row-sharded; the 4 partial outputs per batch are summed on the host
(the all-reduce of the row-parallel projection).

Device kernel layout (per core):
  - host supplies xT = x[b].T  [C, T] in bf16 so all matmuls contract over
    the partition dim with no on-device transposes of x (bf16 end-to-end
    measured 4.1e-3 rel err vs the fp32 reference; softmax normalization
    cancels most of the q/k rounding error).
  - qkvT [576, T] = Wqkv_shard.T @ x.T; the W column order [q0 q2|k0 k2|
    q1 v0|k1 v1|v2] makes every head's q/k slice pair share a base
    partition (a matmul operand requirement) with zero extra M-tiles.
  - scores are computed transposed, S^T [k, q], causal-chunked, so the exp
    (ScalarE) reads PSUM and writes the probability tiles P^T directly in
    the layout the AV matmul wants as its stationary operand:
    out[q,65] += P^T.T @ V'.
  - V' is V with a ones-column appended: column 64 of the AV accumulator
    is then the softmax denominator for free; normalization becomes a
    per-partition reciprocal + tensor_scalar multiply.
  - attn [q,192] is PE-transposed to [192,q] and projected through the
    Wout row-shard; partial [T, C] fp32 is DMA'd out.
"""

import os
import sys

sys.path.insert(0, "/root/.axon_site")
sys.path.insert(0, "/root/.axon_site/_ro/trn_rl_repo")
sys.path.insert(0, "/root/.axon_site/_ro/pypackages")

import numpy as np
import ml_dtypes

import concourse.bass as bass
import concourse.mybir as mybir
import concourse.tile as tile
import concourse.bacc as bacc
from concourse.bass_utils import run_bass_kernel_spmd

B, T, C, H, DH = 2, 2048, 768, 12, 64
G = 4                 # head groups (tensor parallel)
HPG = H // G          # 3 heads per group
CPG = HPG * DH        # 192 output cols per group
NCORES = B * G        # 8

F32 = mybir.dt.float32
F32R = mybir.dt.float32r
BF16 = mybir.dt.bfloat16

NT = T // 128         # 16 t-tiles
NCS = C // 128        # 6 c-strips
SCALE = DH ** -0.5

_COMPILED = {}


def _patch_walrus_ldw_opt():
    """Re-enable walrus's LDWEIGHTS elision: consecutive matmuls with the
    same stationary operand then skip the redundant weight reloads."""
    from concourse import bass_utils
    if getattr(bass_utils, "_ldw_opt_patched", False):
        return
    orig = bass_utils.run_command

    def patched(cmd, **kw):
        cmd = ["--enable-ldw-opt=true" if c == "--enable-ldw-opt=false" else c
               for c in cmd]
        return orig(cmd, **kw)

    bass_utils.run_command = patched
    bass_utils._ldw_opt_patched = True


def _build_nc():
    from contextlib import ExitStack

    import os
    if os.environ.get("LDW_OPT", "0") == "1":
        _patch_walrus_ldw_opt()
    nc = bacc.Bacc("TRN2", debug=False, num_devices=NCORES)

    xt_d = nc.dram_tensor("xt", [C, T], BF16, kind="ExternalInput").ap()
    w_d = nc.dram_tensor("wqkv", [C, 3 * CPG], BF16, kind="ExternalInput").ap()
    wo_d = nc.dram_tensor("wo", [CPG, C], BF16, kind="ExternalInput").ap()
    id_d = nc.dram_tensor("ident", [128, 128], BF16, kind="ExternalInput").ap()
    idf_d = nc.dram_tensor("identf", [65, 65], F32, kind="ExternalInput").ap()
    id2_d = nc.dram_tensor("ident2", [128, 64], BF16, kind="ExternalInput").ap()
    mk_d = nc.dram_tensor("mask", [128, 128], BF16, kind="ExternalInput").ap()
    out_d = nc.dram_tensor("out", [T, C], F32, kind="ExternalOutput").ap()

    with tile.TileContext(nc) as tc, ExitStack() as ctx:
        _kernel(ctx, tc, nc, xt_d, w_d, wo_d, id_d, idf_d, id2_d, mk_d, out_d)

    nc.compile()
    return nc


def _kernel(ctx, tc, nc, xt_d, w_d, wo_d, id_d, idf_d, id2_d, mk_d, out_d):
    Exp = mybir.ActivationFunctionType.Exp

    # ---- persistent SBUF tensors -------------------------------------
    persist = ctx.enter_context(tc.tile_pool(name="persist", bufs=1))

    def single(shape, dtype, name):
        return persist.tile(shape, dtype, tag=name, name=name)

    xt_s = [single([128, T], BF16, f"xt{i}") for i in range(NCS)]
    w_s = [single([128, 3 * CPG], BF16, f"w{i}") for i in range(NCS)]
    wo_hi = single([128, C], BF16, "wohi")
    wo_lo = single([CPG - 128, C], BF16, "wolo")
    ident = single([128, 128], BF16, "ident")
    identf = single([65, 65], F32, "identf")
    ident2 = single([128, 64], BF16, "ident2")  # rows 64..127 = I_64
    maskt = single([128, 128], BF16, "maskt")
    # qkvT rows, bf16 tiles (W col order [q0 q2 k0 k2 q1 v0 k1 v1 v2]):
    #   qk_s[0]=[q0|q2] qk_s[1]=[k0|k2] qk_s[2]=[q1|v0] qk_s[3]=[k1|v1]
    qk_s = [single([128, T], BF16, f"qk{i}") for i in range(4)]
    vt2 = single([64, T], BF16, "vt2")          # [v2]
    # V' per head: [128 k-partitions, 16 k-tiles * 65] (col 64 of each 65 = 1.0)
    vp_s = [single([128, NT * 65], BF16, f"vp{h}") for h in range(HPG)]
    # attention output, all q-tiles side by side: q-tile j at cols [CPG*j:)
    ao_all = single([128, NT * CPG], BF16, "aoall")

    for i in range(NCS):
        nc.sync.dma_start(xt_s[i][:, :], xt_d[i * 128:(i + 1) * 128, :])
        nc.sync.dma_start(w_s[i][:, :], w_d[i * 128:(i + 1) * 128, :])
    nc.sync.dma_start(wo_hi[:, :], wo_d[0:128, :])
    nc.sync.dma_start(wo_lo[:, :], wo_d[128:CPG, :])
    nc.sync.dma_start(ident[:, :], id_d[:, :])
    nc.sync.dma_start(identf[:, :], idf_d[:, :])
    nc.sync.dma_start(ident2[:, :], id2_d[:, :])
    nc.sync.dma_start(maskt[:, :], mk_d[:, :])

    # ---- pools -------------------------------------------------------
    # PSUM budget (8 banks): big 2x[128,1024]f32 slots (4 banks) shared by
    # scores / qkv / transposes / proj, plus 4x[128,65]f32 AV accumulators
    # (4 banks) so AV chains stay deep in flight.
    big = ctx.enter_context(tc.tile_pool(name="pbig", bufs=2, space="PSUM"))
    small = ctx.enter_context(tc.tile_pool(name="psm", bufs=4, space="PSUM"))
    ptp = ctx.enter_context(tc.tile_pool(name="ptp", bufs=2))
    atp = ctx.enter_context(tc.tile_pool(name="atp", bufs=2))
    otp = ctx.enter_context(tc.tile_pool(name="otp", bufs=2))
    rcp = ctx.enter_context(tc.tile_pool(name="rcp", bufs=4))

    nmm = [0]

    def psum_big(p, f, dtype=F32):
        pad = [128, 1024] if dtype == F32 else [128, 2048]
        t = big.tile([p, f], dtype, tag="big", name=f"bg{nmm[0]}",
                     padded_shape=pad)
        nmm[0] += 1
        return t

    def psum_small(p, f, dtype=F32):
        pad = [128, 512] if dtype == F32 else [128, 1024]
        t = small.tile([p, f], dtype, tag="sm", name=f"sm{nmm[0]}",
                       padded_shape=pad)
        nmm[0] += 1
        return t

    # ---- interleaved phases 1+2 --------------------------------------
    # Emission order: qkv m=0,1 -> head-0 and head-2 scores (they only
    # need qk_s[0]/qk_s[1]) so ScalarE starts the exp stream ~25us early
    # and more scores run inside the warm-clock window -> remaining qkv
    # m-tiles + V' transposes -> head-1 scores -> AV per head.
    # M-tiles of qkvT rows: 0:[q0 q2] 1:[k0 k2] 2:[q1 v0] 3:[k1 v1] 4:[v2]
    for h in range(HPG):
        nc.vector.memset(vp_s[h][:, :], 1.0)

    def emit_qkv_m(m):
        rows = 64 if m == 4 else 128
        for n4 in range(4):
            ps = psum_big(rows, 512)
            for cs in range(NCS):
                nc.tensor.matmul(
                    ps[:, :],
                    w_s[cs][:, m * 128:m * 128 + rows],
                    xt_s[cs][:, n4 * 512:(n4 + 1) * 512],
                    start=(cs == 0), stop=(cs == NCS - 1),
                )
            eng = nc.vector if (m * 4 + n4) % 2 == 0 else nc.scalar
            if m < 4:
                dst = qk_s[m][:, n4 * 512:(n4 + 1) * 512]
            else:
                dst = vt2[:rows, n4 * 512:(n4 + 1) * 512]
            if eng is nc.vector:
                eng.tensor_copy(dst, ps[:, :])
            else:
                eng.copy(dst, ps[:, :])

    def emit_vprime(h):
        for i in range(NT):
            tp = psum_small(128, 64, BF16)
            if h < 2:
                nc.tensor.transpose(
                    tp[:, :],
                    qk_s[2 + h][64:128, i * 128:(i + 1) * 128],
                    ident2[64:128, :],
                )
            else:
                nc.tensor.transpose(
                    tp[:, :],
                    vt2[0:64, i * 128:(i + 1) * 128],
                    ident[0:64, 0:64],
                )
            nc.vector.tensor_copy(vp_s[h][:, 65 * i:65 * i + 64], tp[:, :])

    # q/k slices inside qk_s: head h -> (tile, partition offset)
    q_loc = [(0, 0), (2, 0), (0, 64)]
    k_loc = [(1, 0), (3, 0), (1, 64)]

    pt_all = [[None] * NT for _ in range(HPG)]

    def emit_scores(h):
        qt, qp = q_loc[h]
        kt, kp = k_loc[h]
        qT = qk_s[qt][qp:qp + 64, :]
        kT = qk_s[kt][kp:kp + 64, :]
        for i in range(NT):
            qlen = T - 128 * i
            pti = single([128, qlen], BF16, f"pth{h}i{i}")
            pt_all[h][i] = pti
            q0 = 128 * i
            for c0 in range(0, qlen, 1024):
                L = min(1024, qlen - c0)
                sc = psum_big(128, L)
                for s0 in range(0, L, 512):
                    sl = min(512, L - s0)
                    nc.tensor.matmul(
                        sc[:, s0:s0 + sl],
                        kT[:, i * 128:(i + 1) * 128],
                        qT[:, q0 + c0 + s0:q0 + c0 + s0 + sl],
                        start=True, stop=True,
                    )
                nc.scalar.activation(pti[:, c0:c0 + L], sc[:, :L], Exp,
                                     scale=SCALE)
            # zero the upper-triangular (k > q) part of the diagonal block
            nc.vector.tensor_mul(pti[:, 0:128], pti[:, 0:128], maskt[:, :])

    def emit_av(h):
        for j in range(NT):
            po = psum_small(128, 65)
            for i in range(j + 1):
                nc.tensor.matmul(
                    po[:, :],
                    pt_all[h][i][:, (j - i) * 128:(j - i + 1) * 128],
                    vp_s[h][:, 65 * i:65 * i + 65],
                    start=(i == 0), stop=(i == j),
                )
            rec = rcp.tile([128, 1], F32, tag="rc", name=f"rc{h}_{j}")
            nc.vector.reciprocal(rec[:, :], po[:, 64:65])
            nc.vector.tensor_scalar_mul(
                ao_all[:, CPG * j + 64 * h:CPG * j + 64 * h + 64],
                po[:, 0:64], rec[:, :])

    emit_qkv_m(0)
    emit_qkv_m(1)
    emit_scores(0)
    emit_qkv_m(2)
    emit_vprime(0)
    emit_scores(2)
    emit_qkv_m(3)
    emit_qkv_m(4)
    emit_vprime(1)
    emit_vprime(2)
    emit_scores(1)
    emit_av(0)
    emit_av(2)

    # Last head's AV is fused with the output projection: phase-3 work for
    # q-tile j becomes ready right after chain (h=1, j) and overlaps the
    # rest of the AV sweep instead of tailing the kernel.
    for j in range(NT):
        po = psum_small(128, 65)
        for i in range(j + 1):
            nc.tensor.matmul(
                po[:, :],
                pt_all[1][i][:, (j - i) * 128:(j - i + 1) * 128],
                vp_s[1][:, 65 * i:65 * i + 65],
                start=(i == 0), stop=(i == j),
            )
        rec = rcp.tile([128, 1], F32, tag="rc", name=f"rc1_{j}")
        nc.vector.reciprocal(rec[:, :], po[:, 64:65])
        nc.vector.tensor_scalar_mul(
            ao_all[:, CPG * j + 64:CPG * j + 128], po[:, 0:64], rec[:, :])

        t1 = psum_small(128, 128, BF16)
        nc.tensor.transpose(t1[:, :], ao_all[:, CPG * j:CPG * j + 128],
                            ident[:, :])
        t2 = psum_small(64, 128, BF16)
        nc.tensor.transpose(t2[:, :], ao_all[:, CPG * j + 128:CPG * (j + 1)],
                            ident[:, :])
        a_hi = atp.tile([128, 128], BF16, tag="ahi", name=f"ahi{j}")
        a_lo = atp.tile([64, 128], BF16, tag="alo", name=f"alo{j}")
        nc.vector.tensor_copy(a_hi[:, :], t1[:, :])
        nc.vector.tensor_copy(a_lo[:, :], t2[:, :])

        pr = psum_big(128, C)
        for st, lhs, rhs_w in ((True, a_hi, wo_hi), (False, a_lo, wo_lo)):
            for o0, ln in ((0, 512), (512, 256)):
                nc.tensor.matmul(pr[:, o0:o0 + ln], lhs[:, :],
                                 rhs_w[:, o0:o0 + ln], start=st, stop=not st)
        ot = otp.tile([128, C], F32, tag="ot", name=f"ot{j}")
        if j % 2 == 0:
            nc.vector.tensor_copy(ot[:, :], pr[:, :C])
        else:
            nc.scalar.copy(ot[:, :], pr[:, :C])
        nc.sync.dma_start(out_d[j * 128:(j + 1) * 128, :], ot[:, :])


def get_nc():
    if "nc" not in _COMPILED:
        _COMPILED["nc"] = _build_nc()
    return _COMPILED["nc"]


def make_in_maps(x, Wqkv, Wout):
    """Host-side sharding: one input map per core (core = b*G + g)."""
    x = np.asarray(x, dtype=np.float32)
    Wqkv = np.asarray(Wqkv, dtype=np.float32)
    Wout = np.asarray(Wout, dtype=np.float32)

    ident = np.eye(128, dtype=ml_dtypes.bfloat16)
    identf = np.eye(65, dtype=np.float32)
    ident2 = np.zeros((128, 64), dtype=ml_dtypes.bfloat16)
    ident2[64:128, :] = np.eye(64, dtype=ml_dtypes.bfloat16)
    # mask[k, q] = 1 where k <= q  (valid causal entries of the diag block)
    mask = np.triu(np.ones((128, 128), dtype=np.float32)).astype(
        ml_dtypes.bfloat16)

    in_maps = []
    for b in range(B):
        xt = np.ascontiguousarray(x[b].T).astype(ml_dtypes.bfloat16)
        for g in range(G):
            h0, h1, h2 = (g * HPG + hh for hh in range(HPG))

            def col(kind, hd):
                base = {"q": 0, "k": C, "v": 2 * C}[kind]
                return Wqkv[:, base + hd * DH: base + (hd + 1) * DH]

            # column order matches device tile layout:
            # [q0 q2 | k0 k2 | q1 v0 | k1 v1 | v2]
            wqkv = np.concatenate([
                col("q", h0), col("q", h2),
                col("k", h0), col("k", h2),
                col("q", h1), col("v", h0),
                col("k", h1), col("v", h1),
                col("v", h2),
            ], axis=1).astype(ml_dtypes.bfloat16)
            wo = np.concatenate(
                [Wout[hd * DH:(hd + 1) * DH, :] for hd in (h0, h1, h2)],
                axis=0,
            ).astype(ml_dtypes.bfloat16)
            in_maps.append({
                "xt": xt, "wqkv": np.ascontiguousarray(wqkv),
                "wo": np.ascontiguousarray(wo),
                "ident": ident, "identf": identf, "ident2": ident2,
                "mask": mask,
            })
    return in_maps


def kernel(x, Wqkv, Wout):
    nc = get_nc()
    in_maps = make_in_maps(x, Wqkv, Wout)
    res = run_bass_kernel_spmd(nc, in_maps, list(range(NCORES))).results
    out = np.zeros((B, T, C), dtype=np.float32)
    for b in range(B):
        for g in range(G):
            out[b] += res[b * G + g]["out"]
    return out


if __name__ == "__main__":
    nc = get_nc()
    print("built + compiled ok")
